# revision 13
# baseline (speedup 1.0000x reference)
"""Head-sharded (tensor-parallel) CrossAttention kernel for 8 trn2 NeuronCores.

Problem shapes (hardcoded): B=2, L=2048, QD=1024, H=16, D=64.
Each core owns 2 heads end-to-end (proj -> QK-RMSNorm -> RoPE -> attention
-> output projection partial); the all-reduce over cores happens on host.

All matmuls run in bf16 (1 cycle/row on the PE).  Within each head, q/k
rows are permuted to even|odd blocks (host-side Wq/Wk row permutation) so
the RoPE element pairs live in packed slices - scores are invariant to a
consistent q/k row permutation.  Per-core dataflow:

  Phase P(b): x^T tiles (stationary) x Wqkv^T (moving) -> qkv [bl,384] PSUM.
     Staging copy PSUM->SBUF bf16 (ACT for batch 0, DVE for batch 1 since
     ACT is saturated by A(0) exps); sum-of-squares via Pool mul + DVE
     reduce; rrms via Newton rsqrt on DVE; RoPE on DVE (packed bf16
     views, 2x mode); q-hat/k-hat transposed on PE (bf16) into a shared
     pBig PSUM slot, evacuated by one [128,256] copy into the combined
     qkhT resident; v staged as [bl, v|1] with a ones column so the o^T
     matmul also produces the softmax denominator.
  Phase A(b): scores^T [lk=128, lq=1024] = two N=512 matmuls into a
     2-bank PSUM slot; one wide exp per tile via ACT with per-partition
     scale rrms_k/sqrt(D) (k's norm folded into the exp argument -
     softmax computed without max-subtraction, safe since |scores| < 20).
     Emission is software-pipelined: scores(lk+1)+exp(lk+1) are emitted
     before o(lk) so the PE never head-of-line blocks on the exp.
     o^T accumulates [v|1]^T x expS over lk into two 1-bank PSUM halves;
     row 64 = denominator; halves evacuated to SBUF at the unit boundary
     (freeing PSUM for the next unit) then normalize = DVE recip +
     Pool partition-broadcast + DVE mul.  P(1) tiles are interleaved
     into A(0)'s lk stream; O(0) and half of O(1) into A(1)'s.
  Phase O: out[bl,e] partial = o^T (stationary) x Wproj^T (moving);
     PSUM evacuated to bf16, two bl-tiles per output DMA.  Host sums the
     8 partials in fp32 and adds bproj.
"""

import numpy as np

import concourse.bass as bass
import concourse.tile as tile
from concourse import bacc, mybir
from concourse.bass_utils import run_bass_kernel_spmd
from concourse.masks import make_identity

F32 = mybir.dt.float32
BF16 = mybir.dt.bfloat16
AF = mybir.ActivationFunctionType
ALU = mybir.AluOpType

B, L, QD, H, D = 2, 2048, 1024, 16, 64
INNER = H * D
NCORES = 8
HL = H // NCORES          # heads per core = 2
M = HL * D                # 128 head-dim rows per core
BL = B * L                # 4096
NT = BL // 128            # 32 bl-tiles
NTH = NT // 2             # 16 tiles per batch
CT = QD // 128            # 8 contraction tiles
LQC = 1024                # lq chunk (2 psum banks)
NLQ = L // LQC            # 2
NLK = L // 128            # 16 lk chunks
NSB = 4                   # tiles per P sub-batch; 4 sub-batches per b

_CACHE = {}


def _build_nc():
    nc = bacc.Bacc("TRN2", target_bir_lowering=False, debug=False)

    # x^T tiles, two bl-tiles per outer index (one DMA loads both)
    xt = nc.dram_tensor("xt", [NT // 2, 128, 2, CT, 128], BF16, kind="ExternalInput")
    wqkv = nc.dram_tensor("wqkv", [128, CT, 3 * M], BF16, kind="ExternalInput")
    wproj = nc.dram_tensor("wproj", [M, QD], BF16, kind="ExternalInput")
    coefs = nc.dram_tensor("coefs", [128, NLK, 4 * M], BF16, kind="ExternalInput")
    outp = nc.dram_tensor("outp", [BL, QD], BF16, kind="ExternalOutput")

    with tile.TileContext(nc) as tc:
        with (
            tc.tile_pool(name="res", bufs=1) as res,
            tc.tile_pool(name="xs", bufs=4) as xs,
            tc.tile_pool(name="stg", bufs=3) as stg,
            tc.tile_pool(name="wk", bufs=3) as wk,
            tc.tile_pool(name="rq", bufs=3) as rqp,
            tc.tile_pool(name="nrm", bufs=2) as nrm,
            tc.tile_pool(name="es", bufs=8) as esp,
            tc.tile_pool(name="osb", bufs=2) as osbp,
            tc.tile_pool(name="ob", bufs=2) as obp,
            tc.tile_pool(name="pBig", bufs=3, space="PSUM") as pBig,  # 3 x 2 banks
            tc.tile_pool(name="pO", bufs=2, space="PSUM") as pO,      # 2 x 1 bank
        ):
            # ---- residents ----
            wqkv_sb = res.tile([128, CT, 3 * M], BF16)
            nc.sync.dma_start(out=wqkv_sb, in_=wqkv[:, :, :])
            wproj_sb = res.tile([M, QD], BF16)
            nc.sync.dma_start(out=wproj_sb, in_=wproj[:, :])
            coefs_sb = res.tile([128, NLK, 4 * M], BF16)
            nc.sync.dma_start(out=coefs_sb, in_=coefs[:, :, :])

            # q-hat/k-hat in [m, bl] layout: plane 0 = q, plane 1 = k
            qkhT = res.tile([M, 2, BL], BF16)
            vaug = res.tile([128, NT, 2 * (D + 1)], BF16)
            oT = res.tile([M, BL], BF16)
            rr = res.tile([128, NT, 4], F32)

            ones_f = res.tile([128, NT], F32)
            nc.vector.memset(ones_f, 1.0)
            nc.vector.tensor_copy(
                vaug[:, :, D : D + 1].rearrange("p a b -> p (a b)"), ones_f
            )
            nc.vector.tensor_copy(
                vaug[:, :, 2 * D + 1 : 2 * D + 2].rearrange("p a b -> p (a b)"),
                ones_f,
            )
            ident_f = res.tile([128, 128], F32)
            make_identity(nc, ident_f)
            identb = res.tile([128, 128], BF16)
            nc.vector.tensor_copy(identb, ident_f)
            magic = res.tile([128, 16], mybir.dt.int32)
            nc.vector.memset(magic, 0x5F3759DF)

            # ---- P phase, split into per-subbatch front/back parts so the
            # pieces can be interleaved into the A(0) instruction stream ----

            def P_front_tile(bb, sb, t, qk_st, ssq):
                """DMA (pairwise) + proj + stage + v/ssq for one bl-tile."""
                jj = sb * NSB + t
                j = bb * NTH + jj
                if t % 2 == 0:
                    xt_t = xs.tile([128, 2, CT, 128], BF16, tag="xt", name=f"xt{j}")
                    nc.sync.dma_start(out=xt_t, in_=xt[j // 2, :, :, :, :])
                    P_front_tile.xt_t = xt_t
                xt_t = P_front_tile.xt_t
                ps = pBig.tile([128, LQC], F32, tag="big", name=f"proj{j}")
                for ci in range(CT):
                    nc.tensor.matmul(
                        ps[:, 0 : 3 * M],
                        lhsT=xt_t[:, t % 2, ci, :],
                        rhs=wqkv_sb[:, ci, :],
                        start=(ci == 0),
                        stop=(ci == CT - 1),
                    )
                # stage qk: ACT during P(0) (idle), DVE during P(1)
                if bb == 0:
                    nc.scalar.copy(qk_st[:, t, :], ps[:, 0 : 2 * M])
                else:
                    nc.vector.tensor_copy(qk_st[:, t, :], ps[:, 0 : 2 * M])
                nc.gpsimd.tensor_copy(vaug[:, j, 0:D], ps[:, 2 * M : 2 * M + D])
                nc.gpsimd.tensor_copy(
                    vaug[:, j, D + 1 : 2 * D + 1], ps[:, 2 * M + D : 3 * M]
                )
                sqs = wk.tile([128, 2 * M], F32, tag="sqscratch", name=f"sqs{j}")
                nc.gpsimd.tensor_mul(sqs, qk_st[:, t, :], qk_st[:, t, :])
                nc.vector.reduce_sum(
                    out=ssq[:, t, :].rearrange("p (a b) -> p a b", b=1),
                    in_=sqs.rearrange("p (a b) -> p a b", a=4),
                    axis=mybir.AxisListType.X,
                )

            def P_newton(bb, sb, ssq):
                """rrms for 4 tiles via Newton rsqrt on DVE.

                rr cols 0:2 = rrms_q; cols 2:4 = rrms_k/sqrt(D), folded
                into the exp scale."""
                j0 = bb * NTH + sb * NSB
                rrs = rr[:, j0 : j0 + NSB, :]
                nx = wk.tile([128, NSB, 4], F32, tag="nx")
                nc.vector.tensor_scalar(
                    out=nx[:, :, 0:2], in0=ssq[:, :, 0:2],
                    scalar1=1.0 / D, scalar2=1e-6, op0=ALU.mult, op1=ALU.add,
                )
                nc.vector.tensor_scalar(
                    out=nx[:, :, 2:4], in0=ssq[:, :, 2:4],
                    scalar1=1.0, scalar2=float(D) * 1e-6, op0=ALU.mult, op1=ALU.add,
                )
                sh = wk.tile([128, NSB, 4], mybir.dt.int32, tag="nsh")
                nc.vector.tensor_scalar(
                    out=sh, in0=nx.bitcast(mybir.dt.int32), scalar1=1,
                    scalar2=None, op0=ALU.logical_shift_right,
                )
                nc.vector.tensor_tensor(
                    out=rrs.bitcast(mybir.dt.int32),
                    in0=magic[:, 0 : NSB * 4].rearrange("p (a b) -> p a b", b=4),
                    in1=sh,
                    op=ALU.subtract,
                )
                ht = wk.tile([128, NSB, 4], F32, tag="nht")
                for _ in range(2):  # y *= 1.5 - 0.5*x*y*y
                    nc.vector.tensor_mul(ht, nx, rrs)
                    nc.vector.tensor_mul(ht, ht, rrs)
                    nc.vector.tensor_scalar(
                        out=ht, in0=ht, scalar1=-0.5, scalar2=1.5,
                        op0=ALU.mult, op1=ALU.add,
                    )
                    nc.vector.tensor_mul(rrs, rrs, ht)

            def P_back_tile(bb, sb, t, qk_st):
                """normalize q + RoPE + PE transpose + evac for one tile."""
                jj = sb * NSB + t
                j = bb * NTH + jj
                for g in range(2):  # normalize q in place (bf16)
                    nc.gpsimd.tensor_scalar_mul(
                        qk_st[:, t, g * D : (g + 1) * D],
                        qk_st[:, t, g * D : (g + 1) * D],
                        rr[:, j, g : g + 1],
                    )
                # RoPE on packed even|odd halves (all-bf16, 2x DVE)
                qk = rqp.tile([128, 2 * M], BF16, tag="ropeout", name=f"rope{j}")
                t1 = wk.tile([128, M], BF16, tag="ropetmp", name=f"rt{j}")
                src = qk_st[:, t, :].rearrange(
                    "p (g half d2) -> p g half d2", g=4, half=2
                )
                dst = qk.rearrange("p (g half d2) -> p g half d2", g=4, half=2)
                pl = [
                    coefs_sb[:, jj, i * M : (i + 1) * M].rearrange(
                        "p (g d2) -> p g d2", g=4
                    )
                    for i in range(4)
                ]
                t1v = t1.rearrange("p (g d2) -> p g d2", g=4)
                ev, od = src[:, :, 0, :], src[:, :, 1, :]
                nc.vector.tensor_mul(dst[:, :, 0, :], ev, pl[0])
                nc.vector.tensor_mul(t1v, od, pl[1])
                nc.vector.tensor_add(dst[:, :, 0, :], dst[:, :, 0, :], t1v)
                nc.vector.tensor_mul(dst[:, :, 1, :], ev, pl[2])
                nc.vector.tensor_mul(t1v, od, pl[3])
                nc.vector.tensor_add(dst[:, :, 1, :], dst[:, :, 1, :], t1v)

                # move q-hat/k-hat into [m, bl] layout via DMA transpose
                # (no PSUM, no PE, no evac copies; HWDGE has headroom)
                nc.sync.dma_start_transpose(
                    qkhT[:, 0, j * 128 : (j + 1) * 128], qk[:, 0:M]
                )
                nc.sync.dma_start_transpose(
                    qkhT[:, 1, j * 128 : (j + 1) * 128], qk[:, M : 2 * M]
                )

            # ---- A phase ----

            def emit_A_unit(bb, lq, h, inject):
                """one (batch, lq-chunk, head) attention unit; inject(k) is
                called between lk steps to weave in P/O work."""
                qs = qkhT[
                    h * D : (h + 1) * D, 0,
                    bb * L + lq * LQC : bb * L + (lq + 1) * LQC,
                ]
                poh = [
                    pO.tile([D + 1, 512], F32, tag="o", name=f"po{half}")
                    for half in range(2)
                ]

                def emit_o(lk, es):
                    j = bb * NLK + lk
                    for half in range(2):
                        nc.tensor.matmul(
                            poh[half],
                            lhsT=vaug[:, j, h * (D + 1) : (h + 1) * (D + 1)],
                            rhs=es[:, half * 512 : (half + 1) * 512],
                            start=(lk == 0),
                            stop=(lk == NLK - 1),
                            skip_group_check=True,
                        )

                # software pipeline: emit scores(lk)+exp(lk) OLAG steps
                # before o(lk) so PE never head-of-line blocks on the exp
                # or on the pO slot turnover at unit boundaries.
                OLAG = 3
                es_hist = {}
                for lk in range(NLK):
                    j = bb * NLK + lk
                    pss = pBig.tile([128, LQC], F32, tag="big", name=f"ss{lk}")
                    for half in range(2):
                        nc.tensor.matmul(
                            pss[:, half * 512 : (half + 1) * 512],
                            lhsT=qkhT[
                                h * D : (h + 1) * D, 1,
                                bb * L + lk * 128 : bb * L + (lk + 1) * 128,
                            ],
                            rhs=qs[:, half * 512 : (half + 1) * 512],
                            start=True,
                            stop=True,
                        )
                    es = esp.tile([128, LQC], BF16, tag="es", name=f"es{lk}")
                    nc.scalar.activation(
                        out=es, in_=pss, func=AF.Exp,
                        scale=rr[:, j, 2 + h : 3 + h],
                    )
                    es_hist[lk] = es
                    if lk >= OLAG:
                        emit_o(lk - OLAG, es_hist.pop(lk - OLAG))
                    inject(lk)
                for lk in range(NLK - OLAG, NLK):
                    emit_o(lk, es_hist.pop(lk))

                # boundary: evacuate the two po halves to SBUF (frees PSUM
                # for the next unit), then normalize off-PSUM.
                o_sb = osbp.tile([D + 1, LQC], F32, tag="osb")
                nc.vector.tensor_copy(o_sb[:, 0:512], poh[0])
                nc.gpsimd.tensor_copy(o_sb[:, 512:1024], poh[1])
                rd = nrm.tile([1, LQC], F32, tag="rd")
                nc.vector.reciprocal(rd, o_sb[D : D + 1, :])
                rdb = nrm.tile([D, LQC], F32, tag="rdb")
                nc.gpsimd.partition_broadcast(rdb, rd)
                nc.vector.tensor_mul(
                    oT[
                        h * D : (h + 1) * D,
                        bb * L + lq * LQC : bb * L + (lq + 1) * LQC,
                    ],
                    o_sb[0:D, :],
                    rdb,
                )

            # ---- O phase: two bl-tiles per chunk, one output DMA ----

            def emit_O_pair(jp, tail):
                ob = obp.tile([128, 2, QD], BF16, tag="ob", name=f"ob{jp}")
                for u in range(2):
                    j = jp * 2 + u
                    ps = pBig.tile([128, LQC], F32, tag="big", name=f"op{j}")
                    for eo in range(2):
                        nc.tensor.matmul(
                            ps[:, eo * 512 : (eo + 1) * 512],
                            lhsT=oT[:, j * 128 : (j + 1) * 128],
                            rhs=wproj_sb[:, eo * 512 : (eo + 1) * 512],
                            start=True,
                            stop=True,
                        )
                    if tail and u == 0:
                        nc.scalar.copy(ob[:, u, :], ps)
                    elif tail:
                        nc.vector.tensor_copy(ob[:, u, :], ps)
                    elif u == 0:
                        nc.vector.tensor_copy(ob[:, u, :], ps)
                    else:
                        nc.gpsimd.tensor_copy(ob[:, u, :], ps)
                nc.sync.dma_start(
                    out=outp[jp * 256 : (jp + 1) * 256, :],
                    in_=ob.rearrange("p a b -> p (a b)"),
                )

            # ---- emission schedule ----

            # P(0) standalone
            for sb in range(4):
                qk_st = stg.tile([128, NSB, 2 * M], BF16, tag="stage", name=f"st{sb}")
                ssq = stg.tile([128, NSB, 4], F32, tag="ssq", name=f"sq{sb}")
                for t in range(NSB):
                    P_front_tile(0, sb, t, qk_st, ssq)
                P_newton(0, sb, ssq)
                for t in range(NSB):
                    P_back_tile(0, sb, t, qk_st)

            # A(0) with P(1) interleaved: unit u carries subbatch u of P(1)
            unit_idx = 0
            for lq in range(NLQ):
                for h in range(HL):
                    sb = unit_idx
                    qk_st = stg.tile(
                        [128, NSB, 2 * M], BF16, tag="stage", name=f"st1{sb}"
                    )
                    ssq = stg.tile([128, NSB, 4], F32, tag="ssq", name=f"sq1{sb}")
                    state = {"done": 0}

                    def inject(lk, sb=sb, qk_st=qk_st, ssq=ssq, state=state):
                        # front parts at lk 0,3,6,9; newton at 10;
                        # back parts spread at lk 11..14
                        if lk in (0, 3, 6, 9):
                            P_front_tile(1, sb, state["done"], qk_st, ssq)
                            state["done"] += 1
                        elif lk == 10:
                            P_newton(1, sb, ssq)
                        elif lk in (11, 12, 13, 14):
                            P_back_tile(1, sb, lk - 11, qk_st)

                    emit_A_unit(0, lq, h, inject)
                    unit_idx += 1

            # A(1) with O(0) and O(1,lq0) interleaved.
            # O pairs: batch0 -> jp 0..7, batch1 lq0 -> jp 8..11, tail 12..15.
            opairs = [[0, 1], [2, 3], [4, 5, 8], [6, 7, 9]]
            unit_idx = 0
            for lq in range(NLQ):
                for h in range(HL):
                    plan = opairs[unit_idx]

                    def inject(lk, plan=plan, unit=unit_idx):
                        # O(1,lq0) pairs only become ready after unit 1
                        if lk in (3, 8, 13) and plan:
                            jp = plan[0]
                            if jp < 8 or unit >= 2:
                                emit_O_pair(jp, tail=False)
                                plan.pop(0)

                    emit_A_unit(1, lq, h, inject)
                    unit_idx += 1

            # tail: O(1,lq0) leftovers + O(1,lq1)
            for jp in [10, 11, 12, 13, 14, 15]:
                emit_O_pair(jp, tail=True)

    nc.compile()
    return nc


def _prep_inputs(x, pe, Wq, Wkv, Wproj, q_scale, k_scale):
    import ml_dtypes

    bf16 = ml_dtypes.bfloat16

    x = np.asarray(x, np.float32)
    xT = np.ascontiguousarray(x.reshape(BL, QD).T)                    # [QD, BL]
    xtt = (
        xT.reshape(CT, 128, NT // 2, 2, 128)
        .transpose(2, 1, 3, 0, 4)
        .astype(bf16)
    )                                                                 # [NT/2, p, 2, CT, n]
    xtt = np.ascontiguousarray(xtt)

    pe = np.asarray(pe, np.float32)[0, 0]                             # [L, 32, 2, 2]
    qs, ks = np.asarray(q_scale, np.float32), np.asarray(k_scale, np.float32)

    def planes(scale):
        se, so = scale[0::2], scale[1::2]
        return (
            pe[:, :, 0, 0] * se[None, :],
            pe[:, :, 0, 1] * so[None, :],
            pe[:, :, 1, 0] * se[None, :],
            pe[:, :, 1, 1] * so[None, :],
        )

    pq, pk = planes(qs), planes(ks)
    coefs = np.empty((L, 4, 4, 32), np.float32)                       # [l, plane, grp, d2]
    for p_i in range(4):
        coefs[:, p_i, 0] = pq[p_i]
        coefs[:, p_i, 1] = pq[p_i]
        coefs[:, p_i, 2] = pk[p_i]
        coefs[:, p_i, 3] = pk[p_i]
    # on-chip resident layout: [128 (l within lk-tile), NLK, 4M]
    coefs = np.ascontiguousarray(
        coefs.reshape(NLK, 128, 4 * M).transpose(1, 0, 2)
    ).astype(bf16)

    # within-head even|odd q/k row permutation (scores invariant; makes
    # RoPE element pairs contiguous on-chip)
    perm = np.concatenate(
        [h * D + np.concatenate([np.arange(0, D, 2), np.arange(1, D, 2)])
         for h in range(HL)]
    )

    Wq = np.asarray(Wq, np.float32)
    Wkv = np.asarray(Wkv, np.float32)
    Wproj = np.asarray(Wproj, np.float32)
    Wk_full, Wv_full = Wkv[:INNER], Wkv[INNER:]

    in_maps = []
    for c in range(NCORES):
        r0, r1 = c * M, (c + 1) * M
        wqkv_c = np.concatenate(
            [Wq[r0:r1][perm], Wk_full[r0:r1][perm], Wv_full[r0:r1]], axis=0
        )
        wqkv_t = np.ascontiguousarray(
            wqkv_c.T.reshape(CT, 128, 3 * M).transpose(1, 0, 2)
        ).astype(bf16)                                                # [128, CT, 3M]
        wproj_c = np.ascontiguousarray(Wproj[:, r0:r1].T).astype(bf16)  # [M, QD]
        in_maps.append(
            {"xt": xtt, "wqkv": wqkv_t, "wproj": wproj_c, "coefs": coefs}
        )
    return in_maps


def kernel(x, pe, Wq, Wkv, Wproj, bproj, q_scale, k_scale):
    if "nc" not in _CACHE:
        _CACHE["nc"] = _build_nc()
    nc = _CACHE["nc"]
    in_maps = _prep_inputs(x, pe, Wq, Wkv, Wproj, q_scale, k_scale)
    res = run_bass_kernel_spmd(nc, in_maps, core_ids=list(range(NCORES)))
    acc = np.zeros((BL, QD), np.float32)
    for c in range(NCORES):
        acc += res.results[c]["outp"].astype(np.float32)
    acc += np.asarray(bproj, np.float32)[None, :]
    return acc.reshape(B, L, QD)


# revision 14
# speedup vs baseline: 1.1115x; 1.1115x over previous
"""Head-sharded (tensor-parallel) CrossAttention kernel for 8 trn2 NeuronCores.

Problem shapes (hardcoded): B=2, L=2048, QD=1024, H=16, D=64.
Each core owns 2 heads end-to-end (proj -> QK-RMSNorm -> RoPE -> attention
-> output projection partial); the all-reduce over cores happens on host.

All matmuls run in bf16 (1 cycle/row on the PE).  Within each head, q/k
rows are permuted to even|odd blocks (host-side Wq/Wk row permutation) so
the RoPE element pairs live in packed slices - scores are invariant to a
consistent q/k row permutation.  Per-core dataflow:

  Phase P(b): x^T tiles (stationary) x Wqkv^T (moving) -> qkv [bl,384] PSUM.
     Staging copy PSUM->SBUF bf16 (ACT for batch 0, DVE for batch 1 since
     ACT is saturated by A(0) exps); sum-of-squares via Pool mul + DVE
     reduce; rrms via Newton rsqrt on DVE; RoPE on DVE (packed bf16
     views, 2x mode); q-hat/k-hat transposed on PE (bf16) into a shared
     pBig PSUM slot, evacuated by one [128,256] copy into the combined
     qkhT resident; v staged as [bl, v|1] with a ones column so the o^T
     matmul also produces the softmax denominator.
  Phase A(b): scores^T [lk=128, lq=1024] = two N=512 matmuls into a
     2-bank PSUM slot; one wide exp per tile via ACT with per-partition
     scale rrms_k/sqrt(D) (k's norm folded into the exp argument -
     softmax computed without max-subtraction, safe since |scores| < 20).
     Emission is software-pipelined: scores(lk+1)+exp(lk+1) are emitted
     before o(lk) so the PE never head-of-line blocks on the exp.
     o^T accumulates [v|1]^T x expS over lk into two 1-bank PSUM halves;
     row 64 = denominator; halves evacuated to SBUF at the unit boundary
     (freeing PSUM for the next unit) then normalize = DVE recip +
     Pool partition-broadcast + DVE mul.  P(1) tiles are interleaved
     into A(0)'s lk stream; O(0) and half of O(1) into A(1)'s.
  Phase O: out[bl,e] partial = o^T (stationary) x Wproj^T (moving);
     PSUM evacuated to bf16, two bl-tiles per output DMA.  Host sums the
     8 partials in fp32 and adds bproj.
"""

import numpy as np

import concourse.bass as bass
import concourse.tile as tile
from concourse import bacc, mybir
from concourse.bass_utils import run_bass_kernel_spmd
from concourse.masks import make_identity

F32 = mybir.dt.float32
BF16 = mybir.dt.bfloat16
AF = mybir.ActivationFunctionType
ALU = mybir.AluOpType

B, L, QD, H, D = 2, 2048, 1024, 16, 64
INNER = H * D
NCORES = 8
HL = H // NCORES          # heads per core = 2
M = HL * D                # 128 head-dim rows per core
BL = B * L                # 4096
NT = BL // 128            # 32 bl-tiles
NTH = NT // 2             # 16 tiles per batch
CT = QD // 128            # 8 contraction tiles
LQC = 1024                # lq chunk (2 psum banks)
NLQ = L // LQC            # 2
NLK = L // 128            # 16 lk chunks
NSB = 4                   # tiles per P sub-batch; 4 sub-batches per b

_CACHE = {}


def _build_nc():
    nc = bacc.Bacc("TRN2", target_bir_lowering=False, debug=False)

    # x^T tiles, two bl-tiles per outer index (one DMA loads both)
    xt = nc.dram_tensor("xt", [NT // 2, 128, 2, CT, 128], BF16, kind="ExternalInput")
    wqkv = nc.dram_tensor("wqkv", [128, CT, 3 * M], BF16, kind="ExternalInput")
    wproj = nc.dram_tensor("wproj", [M, QD], BF16, kind="ExternalInput")
    coefs = nc.dram_tensor("coefs", [128, NLK, 4 * M], BF16, kind="ExternalInput")
    outp = nc.dram_tensor("outp", [BL, QD], BF16, kind="ExternalOutput")

    with tile.TileContext(nc) as tc:
        with (
            tc.tile_pool(name="res", bufs=1) as res,
            tc.tile_pool(name="xs", bufs=4) as xs,
            tc.tile_pool(name="stg", bufs=3) as stg,
            tc.tile_pool(name="wk", bufs=3) as wk,
            tc.tile_pool(name="rq", bufs=3) as rqp,
            tc.tile_pool(name="nrm", bufs=2) as nrm,
            tc.tile_pool(name="es", bufs=8) as esp,
            tc.tile_pool(name="osb", bufs=2) as osbp,
            tc.tile_pool(name="ob", bufs=2) as obp,
            tc.tile_pool(name="pBig", bufs=3, space="PSUM") as pBig,  # 3 x 2 banks
            tc.tile_pool(name="pO", bufs=2, space="PSUM") as pO,      # 2 x 1 bank
        ):
            # ---- residents ----
            wqkv_sb = res.tile([128, CT, 3 * M], BF16)
            nc.sync.dma_start(out=wqkv_sb, in_=wqkv[:, :, :])
            wproj_sb = res.tile([M, QD], BF16)
            nc.sync.dma_start(out=wproj_sb, in_=wproj[:, :])
            coefs_sb = res.tile([128, NLK, 4 * M], BF16)
            nc.sync.dma_start(out=coefs_sb, in_=coefs[:, :, :])

            # q-hat/k-hat in [m, bl] layout: plane 0 = q, plane 1 = k
            qkhT = res.tile([M, 2, BL], BF16)
            vaug = res.tile([128, NT, 2 * (D + 1)], BF16)
            oT = res.tile([M, BL], BF16)
            rr = res.tile([128, NT, 4], F32)

            ones_f = res.tile([128, NT], F32)
            nc.vector.memset(ones_f, 1.0)
            nc.vector.tensor_copy(
                vaug[:, :, D : D + 1].rearrange("p a b -> p (a b)"), ones_f
            )
            nc.vector.tensor_copy(
                vaug[:, :, 2 * D + 1 : 2 * D + 2].rearrange("p a b -> p (a b)"),
                ones_f,
            )
            ident_f = res.tile([128, 128], F32)
            make_identity(nc, ident_f)
            identb = res.tile([128, 128], BF16)
            nc.vector.tensor_copy(identb, ident_f)
            magic = res.tile([128, 16], mybir.dt.int32)
            nc.vector.memset(magic, 0x5F3759DF)

            # ---- P phase, split into per-subbatch front/back parts so the
            # pieces can be interleaved into the A(0) instruction stream ----

            def P_front_tile(bb, sb, t, qk_st, ssq):
                """DMA (pairwise) + proj + stage + v/ssq for one bl-tile."""
                jj = sb * NSB + t
                j = bb * NTH + jj
                if t % 2 == 0:
                    xt_t = xs.tile([128, 2, CT, 128], BF16, tag="xt", name=f"xt{j}")
                    nc.sync.dma_start(out=xt_t, in_=xt[j // 2, :, :, :, :])
                    P_front_tile.xt_t = xt_t
                xt_t = P_front_tile.xt_t
                ps = pBig.tile([128, LQC], F32, tag="big", name=f"proj{j}")
                for ci in range(CT):
                    nc.tensor.matmul(
                        ps[:, 0 : 3 * M],
                        lhsT=xt_t[:, t % 2, ci, :],
                        rhs=wqkv_sb[:, ci, :],
                        start=(ci == 0),
                        stop=(ci == CT - 1),
                    )
                # stage qk: ACT during P(0) (idle), DVE during P(1)
                if bb == 0:
                    nc.scalar.copy(qk_st[:, t, :], ps[:, 0 : 2 * M])
                else:
                    nc.vector.tensor_copy(qk_st[:, t, :], ps[:, 0 : 2 * M])
                nc.gpsimd.tensor_copy(vaug[:, j, 0:D], ps[:, 2 * M : 2 * M + D])
                nc.gpsimd.tensor_copy(
                    vaug[:, j, D + 1 : 2 * D + 1], ps[:, 2 * M + D : 3 * M]
                )
                sqs = wk.tile([128, 2 * M], F32, tag="sqscratch", name=f"sqs{j}")
                nc.gpsimd.tensor_mul(sqs, qk_st[:, t, :], qk_st[:, t, :])
                nc.vector.reduce_sum(
                    out=ssq[:, t, :].rearrange("p (a b) -> p a b", b=1),
                    in_=sqs.rearrange("p (a b) -> p a b", a=4),
                    axis=mybir.AxisListType.X,
                )

            def P_newton(bb, sb, ssq):
                """rrms for 4 tiles via Newton rsqrt on DVE.

                rr cols 0:2 = rrms_q; cols 2:4 = rrms_k/sqrt(D), folded
                into the exp scale."""
                j0 = bb * NTH + sb * NSB
                rrs = rr[:, j0 : j0 + NSB, :]
                nx = wk.tile([128, NSB, 4], F32, tag="nx")
                nc.vector.tensor_scalar(
                    out=nx[:, :, 0:2], in0=ssq[:, :, 0:2],
                    scalar1=1.0 / D, scalar2=1e-6, op0=ALU.mult, op1=ALU.add,
                )
                nc.vector.tensor_scalar(
                    out=nx[:, :, 2:4], in0=ssq[:, :, 2:4],
                    scalar1=1.0, scalar2=float(D) * 1e-6, op0=ALU.mult, op1=ALU.add,
                )
                sh = wk.tile([128, NSB, 4], mybir.dt.int32, tag="nsh")
                nc.vector.tensor_scalar(
                    out=sh, in0=nx.bitcast(mybir.dt.int32), scalar1=1,
                    scalar2=None, op0=ALU.logical_shift_right,
                )
                nc.vector.tensor_tensor(
                    out=rrs.bitcast(mybir.dt.int32),
                    in0=magic[:, 0 : NSB * 4].rearrange("p (a b) -> p a b", b=4),
                    in1=sh,
                    op=ALU.subtract,
                )
                ht = wk.tile([128, NSB, 4], F32, tag="nht")
                for _ in range(2):  # y *= 1.5 - 0.5*x*y*y
                    nc.vector.tensor_mul(ht, nx, rrs)
                    nc.vector.tensor_mul(ht, ht, rrs)
                    nc.vector.tensor_scalar(
                        out=ht, in0=ht, scalar1=-0.5, scalar2=1.5,
                        op0=ALU.mult, op1=ALU.add,
                    )
                    nc.vector.tensor_mul(rrs, rrs, ht)

            def P_back_tile(bb, sb, t, qk_st):
                """normalize q + RoPE + PE transpose + evac for one tile."""
                jj = sb * NSB + t
                j = bb * NTH + jj
                for g in range(2):  # normalize q in place (bf16)
                    nc.gpsimd.tensor_scalar_mul(
                        qk_st[:, t, g * D : (g + 1) * D],
                        qk_st[:, t, g * D : (g + 1) * D],
                        rr[:, j, g : g + 1],
                    )
                # RoPE on packed even|odd halves (all-bf16, 2x DVE)
                qk = rqp.tile([128, 2 * M], BF16, tag="ropeout", name=f"rope{j}")
                t1 = wk.tile([128, M], BF16, tag="ropetmp", name=f"rt{j}")
                src = qk_st[:, t, :].rearrange(
                    "p (g half d2) -> p g half d2", g=4, half=2
                )
                dst = qk.rearrange("p (g half d2) -> p g half d2", g=4, half=2)
                pl = [
                    coefs_sb[:, jj, i * M : (i + 1) * M].rearrange(
                        "p (g d2) -> p g d2", g=4
                    )
                    for i in range(4)
                ]
                t1v = t1.rearrange("p (g d2) -> p g d2", g=4)
                ev, od = src[:, :, 0, :], src[:, :, 1, :]
                nc.vector.tensor_mul(dst[:, :, 0, :], ev, pl[0])
                nc.vector.tensor_mul(t1v, od, pl[1])
                nc.vector.tensor_add(dst[:, :, 0, :], dst[:, :, 0, :], t1v)
                nc.vector.tensor_mul(dst[:, :, 1, :], ev, pl[2])
                nc.vector.tensor_mul(t1v, od, pl[3])
                nc.vector.tensor_add(dst[:, :, 1, :], dst[:, :, 1, :], t1v)

                # transpose q and k into a pO slot (shares the "o" tag so it
                # costs no extra PSUM banks)
                pqk = pO.tile([128, 256], BF16, tag="o", name=f"tr{j}")
                nc.tensor.transpose(pqk[:, 0:128], qk[:, 0:M], identb)
                nc.tensor.transpose(pqk[:, 128:256], qk[:, M : 2 * M], identb)
                if bb == 0:
                    nc.scalar.copy(
                        qkhT[:, :, j * 128 : (j + 1) * 128], pqk[:, 0:256]
                    )
                else:
                    nc.vector.tensor_copy(
                        qkhT[:, :, j * 128 : (j + 1) * 128], pqk[:, 0:256]
                    )

            # ---- A phase ----

            def emit_A_unit(bb, lq, h, inject):
                """one (batch, lq-chunk, head) attention unit; inject(k) is
                called between lk steps to weave in P/O work."""
                qs = qkhT[
                    h * D : (h + 1) * D, 0,
                    bb * L + lq * LQC : bb * L + (lq + 1) * LQC,
                ]
                poh = [
                    pO.tile([D + 1, 512], F32, tag="o", name=f"po{half}")
                    for half in range(2)
                ]

                def emit_o(lk, es):
                    j = bb * NLK + lk
                    for half in range(2):
                        nc.tensor.matmul(
                            poh[half],
                            lhsT=vaug[:, j, h * (D + 1) : (h + 1) * (D + 1)],
                            rhs=es[:, half * 512 : (half + 1) * 512],
                            start=(lk == 0),
                            stop=(lk == NLK - 1),
                            skip_group_check=True,
                        )

                # software pipeline: emit scores(lk)+exp(lk) OLAG steps
                # before o(lk) so PE never head-of-line blocks on the exp
                # or on the pO slot turnover at unit boundaries.
                OLAG = 3
                es_hist = {}
                for lk in range(NLK):
                    j = bb * NLK + lk
                    pss = pBig.tile([128, LQC], F32, tag="big", name=f"ss{lk}")
                    for half in range(2):
                        nc.tensor.matmul(
                            pss[:, half * 512 : (half + 1) * 512],
                            lhsT=qkhT[
                                h * D : (h + 1) * D, 1,
                                bb * L + lk * 128 : bb * L + (lk + 1) * 128,
                            ],
                            rhs=qs[:, half * 512 : (half + 1) * 512],
                            start=True,
                            stop=True,
                        )
                    es = esp.tile([128, LQC], BF16, tag="es", name=f"es{lk}")
                    nc.scalar.activation(
                        out=es, in_=pss, func=AF.Exp,
                        scale=rr[:, j, 2 + h : 3 + h],
                    )
                    es_hist[lk] = es
                    if lk >= OLAG:
                        emit_o(lk - OLAG, es_hist.pop(lk - OLAG))
                    inject(lk)
                for lk in range(NLK - OLAG, NLK):
                    emit_o(lk, es_hist.pop(lk))

                # boundary: evacuate the two po halves to SBUF (frees PSUM
                # for the next unit), then normalize off-PSUM.
                o_sb = osbp.tile([D + 1, LQC], F32, tag="osb")
                nc.vector.tensor_copy(o_sb[:, 0:512], poh[0])
                nc.gpsimd.tensor_copy(o_sb[:, 512:1024], poh[1])
                rd = nrm.tile([1, LQC], F32, tag="rd")
                nc.vector.reciprocal(rd, o_sb[D : D + 1, :])
                rdb = nrm.tile([D, LQC], F32, tag="rdb")
                nc.gpsimd.partition_broadcast(rdb, rd)
                nc.vector.tensor_mul(
                    oT[
                        h * D : (h + 1) * D,
                        bb * L + lq * LQC : bb * L + (lq + 1) * LQC,
                    ],
                    o_sb[0:D, :],
                    rdb,
                )

            # ---- O phase: two bl-tiles per chunk, one output DMA ----

            def emit_O_pair(jp, tail):
                ob = obp.tile([128, 2, QD], BF16, tag="ob", name=f"ob{jp}")
                for u in range(2):
                    j = jp * 2 + u
                    ps = pBig.tile([128, LQC], F32, tag="big", name=f"op{j}")
                    for eo in range(2):
                        nc.tensor.matmul(
                            ps[:, eo * 512 : (eo + 1) * 512],
                            lhsT=oT[:, j * 128 : (j + 1) * 128],
                            rhs=wproj_sb[:, eo * 512 : (eo + 1) * 512],
                            start=True,
                            stop=True,
                        )
                    if tail and u == 0:
                        nc.scalar.copy(ob[:, u, :], ps)
                    elif tail:
                        nc.vector.tensor_copy(ob[:, u, :], ps)
                    elif u == 0:
                        nc.vector.tensor_copy(ob[:, u, :], ps)
                    else:
                        nc.gpsimd.tensor_copy(ob[:, u, :], ps)
                nc.sync.dma_start(
                    out=outp[jp * 256 : (jp + 1) * 256, :],
                    in_=ob.rearrange("p a b -> p (a b)"),
                )

            # ---- emission schedule ----

            # P(0) standalone
            for sb in range(4):
                qk_st = stg.tile([128, NSB, 2 * M], BF16, tag="stage", name=f"st{sb}")
                ssq = stg.tile([128, NSB, 4], F32, tag="ssq", name=f"sq{sb}")
                for t in range(NSB):
                    P_front_tile(0, sb, t, qk_st, ssq)
                P_newton(0, sb, ssq)
                for t in range(NSB):
                    P_back_tile(0, sb, t, qk_st)

            # A(0) with P(1) interleaved: unit u carries subbatch u of P(1)
            unit_idx = 0
            for lq in range(NLQ):
                for h in range(HL):
                    sb = unit_idx
                    qk_st = stg.tile(
                        [128, NSB, 2 * M], BF16, tag="stage", name=f"st1{sb}"
                    )
                    ssq = stg.tile([128, NSB, 4], F32, tag="ssq", name=f"sq1{sb}")
                    state = {"done": 0}

                    def inject(lk, sb=sb, qk_st=qk_st, ssq=ssq, state=state):
                        # front parts at lk 0,3,6,9; newton at 10;
                        # back parts spread at lk 11..14
                        if lk in (0, 3, 6, 9):
                            P_front_tile(1, sb, state["done"], qk_st, ssq)
                            state["done"] += 1
                        elif lk == 10:
                            P_newton(1, sb, ssq)
                        elif lk in (11, 12, 13, 14):
                            P_back_tile(1, sb, lk - 11, qk_st)

                    emit_A_unit(0, lq, h, inject)
                    unit_idx += 1

            # A(1) with O(0) and O(1,lq0) interleaved.
            # O pairs: batch0 -> jp 0..7, batch1 lq0 -> jp 8..11, tail 12..15.
            opairs = [[0, 1], [2, 3], [4, 5, 8], [6, 7, 9]]
            unit_idx = 0
            for lq in range(NLQ):
                for h in range(HL):
                    plan = opairs[unit_idx]

                    def inject(lk, plan=plan, unit=unit_idx):
                        # O(1,lq0) pairs only become ready after unit 1
                        if lk in (3, 8, 13) and plan:
                            jp = plan[0]
                            if jp < 8 or unit >= 2:
                                emit_O_pair(jp, tail=False)
                                plan.pop(0)

                    emit_A_unit(1, lq, h, inject)
                    unit_idx += 1

            # tail: O(1,lq0) leftovers + O(1,lq1)
            for jp in [10, 11, 12, 13, 14, 15]:
                emit_O_pair(jp, tail=True)

    nc.compile()
    return nc


def _prep_inputs(x, pe, Wq, Wkv, Wproj, q_scale, k_scale):
    import ml_dtypes

    bf16 = ml_dtypes.bfloat16

    x = np.asarray(x, np.float32)
    xT = np.ascontiguousarray(x.reshape(BL, QD).T)                    # [QD, BL]
    xtt = (
        xT.reshape(CT, 128, NT // 2, 2, 128)
        .transpose(2, 1, 3, 0, 4)
        .astype(bf16)
    )                                                                 # [NT/2, p, 2, CT, n]
    xtt = np.ascontiguousarray(xtt)

    pe = np.asarray(pe, np.float32)[0, 0]                             # [L, 32, 2, 2]
    qs, ks = np.asarray(q_scale, np.float32), np.asarray(k_scale, np.float32)

    def planes(scale):
        se, so = scale[0::2], scale[1::2]
        return (
            pe[:, :, 0, 0] * se[None, :],
            pe[:, :, 0, 1] * so[None, :],
            pe[:, :, 1, 0] * se[None, :],
            pe[:, :, 1, 1] * so[None, :],
        )

    pq, pk = planes(qs), planes(ks)
    coefs = np.empty((L, 4, 4, 32), np.float32)                       # [l, plane, grp, d2]
    for p_i in range(4):
        coefs[:, p_i, 0] = pq[p_i]
        coefs[:, p_i, 1] = pq[p_i]
        coefs[:, p_i, 2] = pk[p_i]
        coefs[:, p_i, 3] = pk[p_i]
    # on-chip resident layout: [128 (l within lk-tile), NLK, 4M]
    coefs = np.ascontiguousarray(
        coefs.reshape(NLK, 128, 4 * M).transpose(1, 0, 2)
    ).astype(bf16)

    # within-head even|odd q/k row permutation (scores invariant; makes
    # RoPE element pairs contiguous on-chip)
    perm = np.concatenate(
        [h * D + np.concatenate([np.arange(0, D, 2), np.arange(1, D, 2)])
         for h in range(HL)]
    )

    Wq = np.asarray(Wq, np.float32)
    Wkv = np.asarray(Wkv, np.float32)
    Wproj = np.asarray(Wproj, np.float32)
    Wk_full, Wv_full = Wkv[:INNER], Wkv[INNER:]

    in_maps = []
    for c in range(NCORES):
        r0, r1 = c * M, (c + 1) * M
        wqkv_c = np.concatenate(
            [Wq[r0:r1][perm], Wk_full[r0:r1][perm], Wv_full[r0:r1]], axis=0
        )
        wqkv_t = np.ascontiguousarray(
            wqkv_c.T.reshape(CT, 128, 3 * M).transpose(1, 0, 2)
        ).astype(bf16)                                                # [128, CT, 3M]
        wproj_c = np.ascontiguousarray(Wproj[:, r0:r1].T).astype(bf16)  # [M, QD]
        in_maps.append(
            {"xt": xtt, "wqkv": wqkv_t, "wproj": wproj_c, "coefs": coefs}
        )
    return in_maps


def kernel(x, pe, Wq, Wkv, Wproj, bproj, q_scale, k_scale):
    if "nc" not in _CACHE:
        _CACHE["nc"] = _build_nc()
    nc = _CACHE["nc"]
    in_maps = _prep_inputs(x, pe, Wq, Wkv, Wproj, q_scale, k_scale)
    res = run_bass_kernel_spmd(nc, in_maps, core_ids=list(range(NCORES)))
    acc = np.zeros((BL, QD), np.float32)
    for c in range(NCORES):
        acc += res.results[c]["outp"].astype(np.float32)
    acc += np.asarray(bproj, np.float32)[None, :]
    return acc.reshape(B, L, QD)


# revision 19
# speedup vs baseline: 1.1635x; 1.0467x over previous
"""Head-sharded (tensor-parallel) CrossAttention kernel for 8 trn2 NeuronCores.

Problem shapes (hardcoded): B=2, L=2048, QD=1024, H=16, D=64.
Each core owns 2 heads end-to-end (proj -> QK-RMSNorm -> RoPE -> attention
-> output projection partial); the all-reduce over cores happens on host.

All matmuls run in bf16 (1 cycle/row on the PE).  Within each head, q/k
rows are permuted to even|odd blocks (host-side Wq/Wk row permutation) so
the RoPE element pairs live in packed slices - scores are invariant to a
consistent q/k row permutation.  Per-core dataflow:

  Phase P(b): x^T tiles (stationary) x Wqkv^T (moving) -> qkv [bl,384] PSUM.
     Staging copy PSUM->SBUF bf16 (ACT for batch 0, DVE for batch 1 since
     ACT is saturated by A(0) exps); sum-of-squares via Pool mul + DVE
     reduce; rrms via Newton rsqrt on DVE; RoPE on DVE (packed bf16
     views, 2x mode); q-hat/k-hat transposed on PE (bf16) into a shared
     pBig PSUM slot, evacuated by one [128,256] copy into the combined
     qkhT resident; v staged as [bl, v|1] with a ones column so the o^T
     matmul also produces the softmax denominator.
  Phase A(b): scores^T [lk=128, lq=1024] = two N=512 matmuls into a
     2-bank PSUM slot; one wide exp per tile via ACT with per-partition
     scale rrms_k/sqrt(D) (k's norm folded into the exp argument -
     softmax computed without max-subtraction, safe since |scores| < 20).
     Emission is software-pipelined: scores(lk+1)+exp(lk+1) are emitted
     before o(lk) so the PE never head-of-line blocks on the exp.
     o^T accumulates [v|1]^T x expS over lk into two 1-bank PSUM halves;
     row 64 = denominator; halves evacuated to SBUF at the unit boundary
     (freeing PSUM for the next unit) then normalize = DVE recip +
     Pool partition-broadcast + DVE mul.  P(1) tiles are interleaved
     into A(0)'s lk stream; O(0) and half of O(1) into A(1)'s.
  Phase O: out[bl,e] partial = o^T (stationary) x Wproj^T (moving);
     PSUM evacuated to bf16, two bl-tiles per output DMA.  Host sums the
     8 partials in fp32 and adds bproj.
"""

import numpy as np

import concourse.bass as bass
import concourse.tile as tile
from concourse import bacc, mybir
from concourse.bass_utils import run_bass_kernel_spmd
from concourse.masks import make_identity

F32 = mybir.dt.float32
BF16 = mybir.dt.bfloat16
AF = mybir.ActivationFunctionType
ALU = mybir.AluOpType

B, L, QD, H, D = 2, 2048, 1024, 16, 64
INNER = H * D
NCORES = 8
HL = H // NCORES          # heads per core = 2
M = HL * D                # 128 head-dim rows per core
BL = B * L                # 4096
NT = BL // 128            # 32 bl-tiles
NTH = NT // 2             # 16 tiles per batch
CT = QD // 128            # 8 contraction tiles
LQC = 1024                # lq chunk (2 psum banks)
NLQ = L // LQC            # 2
NLK = L // 128            # 16 lk chunks
NSB = 4                   # tiles per P sub-batch; 4 sub-batches per b

_CACHE = {}


def _build_nc():
    nc = bacc.Bacc("TRN2", target_bir_lowering=False, debug=False)

    # x^T tiles, two bl-tiles per outer index (one DMA loads both)
    xt = nc.dram_tensor("xt", [NT // 2, 128, 2, CT, 128], BF16, kind="ExternalInput")
    wqkv = nc.dram_tensor("wqkv", [128, CT, 3 * M], BF16, kind="ExternalInput")
    wproj = nc.dram_tensor("wproj", [M, QD], BF16, kind="ExternalInput")
    coefs = nc.dram_tensor("coefs", [128, NLK, 4 * M], BF16, kind="ExternalInput")
    outp = nc.dram_tensor("outp", [BL, QD], BF16, kind="ExternalOutput")

    with tile.TileContext(nc) as tc:
        with (
            tc.tile_pool(name="res", bufs=1) as res,
            tc.tile_pool(name="xs", bufs=4) as xs,
            tc.tile_pool(name="stg", bufs=3) as stg,
            tc.tile_pool(name="wk", bufs=3) as wk,
            tc.tile_pool(name="rq", bufs=3) as rqp,
            tc.tile_pool(name="nrm", bufs=2) as nrm,
            tc.tile_pool(name="es", bufs=8) as esp,
            tc.tile_pool(name="osb", bufs=2) as osbp,
            tc.tile_pool(name="ob", bufs=3) as obp,
            tc.tile_pool(name="pBig", bufs=3, space="PSUM") as pBig,  # 3 x 2 banks
            tc.tile_pool(name="pO", bufs=2, space="PSUM") as pO,      # 2 x 1 bank
        ):
            # ---- residents (coefs/wproj DMAs issued later: the in-order
            # SP queue + shared DMA device would delay the first x tiles) ----
            wqkv_sb = res.tile([128, CT, 3 * M], BF16)
            nc.sync.dma_start(out=wqkv_sb, in_=wqkv[:, :, :])
            wproj_sb = res.tile([M, QD], BF16)
            coefs_sb = res.tile([128, NLK, 4 * M], BF16)

            # q-hat/k-hat in [m, bl] layout: plane 0 = q, plane 1 = k
            qkhT = res.tile([M, 2, BL], BF16)
            vaug = res.tile([128, NT, 2 * (D + 1)], BF16)
            oT = res.tile([M, BL], BF16)
            rr = res.tile([128, NT, 4], F32)

            ones_f = res.tile([128, NT], F32)
            nc.vector.memset(ones_f, 1.0)
            nc.vector.tensor_copy(
                vaug[:, :, D : D + 1].rearrange("p a b -> p (a b)"), ones_f
            )
            nc.vector.tensor_copy(
                vaug[:, :, 2 * D + 1 : 2 * D + 2].rearrange("p a b -> p (a b)"),
                ones_f,
            )
            ident_f = res.tile([128, 128], F32)
            make_identity(nc, ident_f)
            identb = res.tile([128, 128], BF16)
            nc.vector.tensor_copy(identb, ident_f)
            magic = res.tile([128, 16], mybir.dt.int32)
            nc.vector.memset(magic, 0x5F3759DF)

            # ---- P phase, split into per-subbatch front/back parts so the
            # pieces can be interleaved into the A(0) instruction stream ----

            def P_front_tile(bb, sb, t, qk_st, ssq):
                """DMA (pairwise) + proj + stage + v/ssq for one bl-tile."""
                jj = sb * NSB + t
                j = bb * NTH + jj
                if t % 2 == 0:
                    xt_t = xs.tile([128, 2, CT, 128], BF16, tag="xt", name=f"xt{j}")
                    nc.sync.dma_start(out=xt_t, in_=xt[j // 2, :, :, :, :])
                    P_front_tile.xt_t = xt_t
                xt_t = P_front_tile.xt_t
                ps = pBig.tile([128, LQC], F32, tag="big", name=f"proj{j}")
                for ci in range(CT):
                    nc.tensor.matmul(
                        ps[:, 0 : 3 * M],
                        lhsT=xt_t[:, t % 2, ci, :],
                        rhs=wqkv_sb[:, ci, :],
                        start=(ci == 0),
                        stop=(ci == CT - 1),
                    )
                # stage qk: ACT during P(0) (idle), DVE during P(1)
                if bb == 0:
                    nc.scalar.copy(qk_st[:, t, :], ps[:, 0 : 2 * M])
                else:
                    nc.vector.tensor_copy(qk_st[:, t, :], ps[:, 0 : 2 * M])
                nc.gpsimd.tensor_copy(vaug[:, j, 0:D], ps[:, 2 * M : 2 * M + D])
                nc.gpsimd.tensor_copy(
                    vaug[:, j, D + 1 : 2 * D + 1], ps[:, 2 * M + D : 3 * M]
                )
                sqs = wk.tile([128, 2 * M], F32, tag="sqscratch", name=f"sqs{j}")
                nc.gpsimd.tensor_mul(sqs, qk_st[:, t, :], qk_st[:, t, :])
                nc.vector.reduce_sum(
                    out=ssq[:, t, :].rearrange("p (a b) -> p a b", b=1),
                    in_=sqs.rearrange("p (a b) -> p a b", a=4),
                    axis=mybir.AxisListType.X,
                )

            def P_newton(bb, sb, ssq):
                """rrms for 4 tiles via Newton rsqrt on DVE.

                rr cols 0:2 = rrms_q; cols 2:4 = rrms_k/sqrt(D), folded
                into the exp scale."""
                j0 = bb * NTH + sb * NSB
                rrs = rr[:, j0 : j0 + NSB, :]
                nx = wk.tile([128, NSB, 4], F32, tag="nx")
                nc.vector.tensor_scalar(
                    out=nx[:, :, 0:2], in0=ssq[:, :, 0:2],
                    scalar1=1.0 / D, scalar2=1e-6, op0=ALU.mult, op1=ALU.add,
                )
                nc.vector.tensor_scalar(
                    out=nx[:, :, 2:4], in0=ssq[:, :, 2:4],
                    scalar1=1.0, scalar2=float(D) * 1e-6, op0=ALU.mult, op1=ALU.add,
                )
                sh = wk.tile([128, NSB, 4], mybir.dt.int32, tag="nsh")
                nc.vector.tensor_scalar(
                    out=sh, in0=nx.bitcast(mybir.dt.int32), scalar1=1,
                    scalar2=None, op0=ALU.logical_shift_right,
                )
                nc.vector.tensor_tensor(
                    out=rrs.bitcast(mybir.dt.int32),
                    in0=magic[:, 0 : NSB * 4].rearrange("p (a b) -> p a b", b=4),
                    in1=sh,
                    op=ALU.subtract,
                )
                ht = wk.tile([128, NSB, 4], F32, tag="nht")
                for _ in range(2):  # y *= 1.5 - 0.5*x*y*y
                    nc.vector.tensor_mul(ht, nx, rrs)
                    nc.vector.tensor_mul(ht, ht, rrs)
                    nc.vector.tensor_scalar(
                        out=ht, in0=ht, scalar1=-0.5, scalar2=1.5,
                        op0=ALU.mult, op1=ALU.add,
                    )
                    nc.vector.tensor_mul(rrs, rrs, ht)

            def P_back_tile(bb, sb, t, qk_st):
                """normalize q + RoPE + PE transpose + evac for one tile."""
                jj = sb * NSB + t
                j = bb * NTH + jj
                for g in range(2):  # normalize q in place (bf16)
                    nc.gpsimd.tensor_scalar_mul(
                        qk_st[:, t, g * D : (g + 1) * D],
                        qk_st[:, t, g * D : (g + 1) * D],
                        rr[:, j, g : g + 1],
                    )
                # RoPE on packed even|odd halves (all-bf16, 2x DVE)
                qk = rqp.tile([128, 2 * M], BF16, tag="ropeout", name=f"rope{j}")
                t1 = wk.tile([128, M], BF16, tag="ropetmp", name=f"rt{j}")
                src = qk_st[:, t, :].rearrange(
                    "p (g half d2) -> p g half d2", g=4, half=2
                )
                dst = qk.rearrange("p (g half d2) -> p g half d2", g=4, half=2)
                pl = [
                    coefs_sb[:, jj, i * M : (i + 1) * M].rearrange(
                        "p (g d2) -> p g d2", g=4
                    )
                    for i in range(4)
                ]
                t1v = t1.rearrange("p (g d2) -> p g d2", g=4)
                ev, od = src[:, :, 0, :], src[:, :, 1, :]
                nc.vector.tensor_mul(dst[:, :, 0, :], ev, pl[0])
                nc.vector.tensor_mul(t1v, od, pl[1])
                nc.vector.tensor_add(dst[:, :, 0, :], dst[:, :, 0, :], t1v)
                nc.vector.tensor_mul(dst[:, :, 1, :], ev, pl[2])
                nc.vector.tensor_mul(t1v, od, pl[3])
                nc.vector.tensor_add(dst[:, :, 1, :], dst[:, :, 1, :], t1v)

                # transpose q and k into a pO slot (shares the "o" tag so it
                # costs no extra PSUM banks)
                pqk = pO.tile([128, 256], BF16, tag="o", name=f"tr{j}")
                nc.tensor.transpose(pqk[:, 0:128], qk[:, 0:M], identb)
                nc.tensor.transpose(pqk[:, 128:256], qk[:, M : 2 * M], identb)
                if bb == 0:
                    nc.scalar.copy(
                        qkhT[:, :, j * 128 : (j + 1) * 128], pqk[:, 0:256]
                    )
                else:
                    nc.vector.tensor_copy(
                        qkhT[:, :, j * 128 : (j + 1) * 128], pqk[:, 0:256]
                    )

            # ---- A phase ----

            def emit_A_unit(bb, lq, h, inject):
                """one (batch, lq-chunk, head) attention unit; inject(k) is
                called between lk steps to weave in P/O work."""
                qs = qkhT[
                    h * D : (h + 1) * D, 0,
                    bb * L + lq * LQC : bb * L + (lq + 1) * LQC,
                ]
                poh = [
                    pO.tile([D + 1, 512], F32, tag="o", name=f"po{half}")
                    for half in range(2)
                ]

                def emit_o(lk, es):
                    j = bb * NLK + lk
                    for half in range(2):
                        nc.tensor.matmul(
                            poh[half],
                            lhsT=vaug[:, j, h * (D + 1) : (h + 1) * (D + 1)],
                            rhs=es[:, half * 512 : (half + 1) * 512],
                            start=(lk == 0),
                            stop=(lk == NLK - 1),
                            skip_group_check=True,
                        )

                # software pipeline: emit scores(lk)+exp(lk) OLAG steps
                # before o(lk) so PE never head-of-line blocks on the exp
                # or on the pO slot turnover at unit boundaries.
                OLAG = 3
                es_hist = {}
                for lk in range(NLK):
                    j = bb * NLK + lk
                    pss = pBig.tile([128, LQC], F32, tag="big", name=f"ss{lk}")
                    for half in range(2):
                        nc.tensor.matmul(
                            pss[:, half * 512 : (half + 1) * 512],
                            lhsT=qkhT[
                                h * D : (h + 1) * D, 1,
                                bb * L + lk * 128 : bb * L + (lk + 1) * 128,
                            ],
                            rhs=qs[:, half * 512 : (half + 1) * 512],
                            start=True,
                            stop=True,
                        )
                    es = esp.tile([128, LQC], BF16, tag="es", name=f"es{lk}")
                    nc.scalar.activation(
                        out=es, in_=pss, func=AF.Exp,
                        scale=rr[:, j, 2 + h : 3 + h],
                    )
                    es_hist[lk] = es
                    if lk >= OLAG:
                        emit_o(lk - OLAG, es_hist.pop(lk - OLAG))
                    inject(lk)
                for lk in range(NLK - OLAG, NLK):
                    emit_o(lk, es_hist.pop(lk))

                # boundary: evacuate the two po halves to SBUF (frees PSUM
                # for the next unit), then normalize off-PSUM.
                o_sb = osbp.tile([D + 1, LQC], F32, tag="osb")
                nc.vector.tensor_copy(o_sb[:, 0:512], poh[0])
                nc.gpsimd.tensor_copy(o_sb[:, 512:1024], poh[1])
                rd = nrm.tile([1, LQC], F32, tag="rd")
                nc.vector.reciprocal(rd, o_sb[D : D + 1, :])
                rdb = nrm.tile([D, LQC], F32, tag="rdb")
                nc.gpsimd.partition_broadcast(rdb, rd)
                nc.vector.tensor_mul(
                    oT[
                        h * D : (h + 1) * D,
                        bb * L + lq * LQC : bb * L + (lq + 1) * LQC,
                    ],
                    o_sb[0:D, :],
                    rdb,
                )

            # ---- O phase: two bl-tiles per chunk, one output DMA ----

            def emit_O_pair(jp, tail):
                ob = obp.tile([128, 2, QD], BF16, tag="ob", name=f"ob{jp}")
                for u in range(2):
                    j = jp * 2 + u
                    ps = pBig.tile([128, LQC], F32, tag="big", name=f"op{j}")
                    for eo in range(2):
                        nc.tensor.matmul(
                            ps[:, eo * 512 : (eo + 1) * 512],
                            lhsT=oT[:, j * 128 : (j + 1) * 128],
                            rhs=wproj_sb[:, eo * 512 : (eo + 1) * 512],
                            start=True,
                            stop=True,
                        )
                    if tail and u == 0:
                        nc.scalar.copy(ob[:, u, :], ps)
                    elif tail:
                        nc.vector.tensor_copy(ob[:, u, :], ps)
                    elif u == 0:
                        nc.vector.tensor_copy(ob[:, u, :], ps)
                    else:
                        nc.gpsimd.tensor_copy(ob[:, u, :], ps)
                nc.sync.dma_start(
                    out=outp[jp * 256 : (jp + 1) * 256, :],
                    in_=ob.rearrange("p a b -> p (a b)"),
                )

            # ---- emission schedule ----

            # P(0) standalone
            for sb in range(4):
                qk_st = stg.tile([128, NSB, 2 * M], BF16, tag="stage", name=f"st{sb}")
                ssq = stg.tile([128, NSB, 4], F32, tag="ssq", name=f"sq{sb}")
                for t in range(NSB):
                    P_front_tile(0, sb, t, qk_st, ssq)
                    if sb == 0 and t == 1:
                        # coefs needed from the first P_back (rope) on
                        nc.sync.dma_start(out=coefs_sb, in_=coefs[:, :, :])
                P_newton(0, sb, ssq)
                for t in range(NSB):
                    P_back_tile(0, sb, t, qk_st)
                if sb == 2:
                    # wproj needed from the first O pair (inside A(1))
                    nc.sync.dma_start(out=wproj_sb, in_=wproj[:, :])

            # A(0) with P(1) interleaved: unit u carries subbatch u of P(1)
            unit_idx = 0
            for lq in range(NLQ):
                for h in range(HL):
                    sb = unit_idx
                    qk_st = stg.tile(
                        [128, NSB, 2 * M], BF16, tag="stage", name=f"st1{sb}"
                    )
                    ssq = stg.tile([128, NSB, 4], F32, tag="ssq", name=f"sq1{sb}")
                    state = {"done": 0}

                    def inject(lk, sb=sb, qk_st=qk_st, ssq=ssq, state=state):
                        # front parts at lk 0,3,6,9; newton at 10;
                        # back parts spread at lk 11..14
                        if lk in (0, 3, 6, 9):
                            P_front_tile(1, sb, state["done"], qk_st, ssq)
                            state["done"] += 1
                        elif lk == 10:
                            P_newton(1, sb, ssq)
                        elif lk in (11, 12, 13, 14):
                            P_back_tile(1, sb, lk - 11, qk_st)

                    emit_A_unit(0, lq, h, inject)
                    unit_idx += 1

            # A(1) with O(0) and O(1,lq0) interleaved.
            # O pairs: batch0 -> jp 0..7, batch1 lq0 -> jp 8..11, tail 12..15.
            opairs = [[0, 1, 2], [3, 4, 5], [6, 7, 8], [9, 10, 11]]
            unit_idx = 0
            for lq in range(NLQ):
                for h in range(HL):
                    plan = opairs[unit_idx]

                    def inject(lk, plan=plan, unit=unit_idx):
                        # O(1,lq0) pairs only become ready after unit 1
                        if lk in (3, 8, 13) and plan:
                            jp = plan[0]
                            if jp < 8 or unit >= 2:
                                emit_O_pair(jp, tail=False)
                                plan.pop(0)

                    emit_A_unit(1, lq, h, inject)
                    unit_idx += 1

            # tail: O(1,lq1)
            for jp in [12, 13, 14, 15]:
                emit_O_pair(jp, tail=True)

    nc.compile()
    return nc


def _prep_inputs(x, pe, Wq, Wkv, Wproj, q_scale, k_scale):
    import ml_dtypes

    bf16 = ml_dtypes.bfloat16

    x = np.asarray(x, np.float32)
    xT = np.ascontiguousarray(x.reshape(BL, QD).T)                    # [QD, BL]
    xtt = (
        xT.reshape(CT, 128, NT // 2, 2, 128)
        .transpose(2, 1, 3, 0, 4)
        .astype(bf16)
    )                                                                 # [NT/2, p, 2, CT, n]
    xtt = np.ascontiguousarray(xtt)

    pe = np.asarray(pe, np.float32)[0, 0]                             # [L, 32, 2, 2]
    qs, ks = np.asarray(q_scale, np.float32), np.asarray(k_scale, np.float32)

    def planes(scale):
        se, so = scale[0::2], scale[1::2]
        return (
            pe[:, :, 0, 0] * se[None, :],
            pe[:, :, 0, 1] * so[None, :],
            pe[:, :, 1, 0] * se[None, :],
            pe[:, :, 1, 1] * so[None, :],
        )

    pq, pk = planes(qs), planes(ks)
    coefs = np.empty((L, 4, 4, 32), np.float32)                       # [l, plane, grp, d2]
    for p_i in range(4):
        coefs[:, p_i, 0] = pq[p_i]
        coefs[:, p_i, 1] = pq[p_i]
        coefs[:, p_i, 2] = pk[p_i]
        coefs[:, p_i, 3] = pk[p_i]
    # on-chip resident layout: [128 (l within lk-tile), NLK, 4M]
    coefs = np.ascontiguousarray(
        coefs.reshape(NLK, 128, 4 * M).transpose(1, 0, 2)
    ).astype(bf16)

    # within-head even|odd q/k row permutation (scores invariant; makes
    # RoPE element pairs contiguous on-chip)
    perm = np.concatenate(
        [h * D + np.concatenate([np.arange(0, D, 2), np.arange(1, D, 2)])
         for h in range(HL)]
    )

    Wq = np.asarray(Wq, np.float32)
    Wkv = np.asarray(Wkv, np.float32)
    Wproj = np.asarray(Wproj, np.float32)
    Wk_full, Wv_full = Wkv[:INNER], Wkv[INNER:]

    in_maps = []
    for c in range(NCORES):
        r0, r1 = c * M, (c + 1) * M
        wqkv_c = np.concatenate(
            [Wq[r0:r1][perm], Wk_full[r0:r1][perm], Wv_full[r0:r1]], axis=0
        )
        wqkv_t = np.ascontiguousarray(
            wqkv_c.T.reshape(CT, 128, 3 * M).transpose(1, 0, 2)
        ).astype(bf16)                                                # [128, CT, 3M]
        wproj_c = np.ascontiguousarray(Wproj[:, r0:r1].T).astype(bf16)  # [M, QD]
        in_maps.append(
            {"xt": xtt, "wqkv": wqkv_t, "wproj": wproj_c, "coefs": coefs}
        )
    return in_maps


def kernel(x, pe, Wq, Wkv, Wproj, bproj, q_scale, k_scale):
    if "nc" not in _CACHE:
        _CACHE["nc"] = _build_nc()
    nc = _CACHE["nc"]
    in_maps = _prep_inputs(x, pe, Wq, Wkv, Wproj, q_scale, k_scale)
    res = run_bass_kernel_spmd(nc, in_maps, core_ids=list(range(NCORES)))
    acc = np.zeros((BL, QD), np.float32)
    for c in range(NCORES):
        acc += res.results[c]["outp"].astype(np.float32)
    acc += np.asarray(bproj, np.float32)[None, :]
    return acc.reshape(B, L, QD)
